# revision 1
# baseline (speedup 1.0000x reference)
"""Trainium2 Bass kernel for nn_BlockCrossAttn (block-diagonal attention, E=H=1).

Math per (block b, batch n) pair (256-long vectors q', k', v of the block):
    q' = wq*Q + bq ; k' = wk*K + bk
    soft[q,k] = softmax_k(q'[q] * k'[k])
    out[q] = wvo * (sum_k soft[q,k] * V[k]) + (bvo + bo)
where wvo = wo*wv, bvo = wo*bv (the V/out affine folds into the epilogue
because softmax weights sum to 1).  No max-subtraction: |scores| <= ~27
worst case, exp is safe in fp32.

Sharding: 128 blocks of 256 rows; 16 blocks per core across 8 cores
(fully independent, no collectives).

Per-core device pipeline (512 pairs):
  - PE outer products (contraction dim 1) build S^T[k, q] in PSUM,
    3 pairs per 3-bank group, double buffered.
  - ScalarE exp over [128, 1536] PSUM spans -> E in SBUF.
  - PE reduction matmuls: lhsT = [ones, v] 2-column AP (arbitrary free
    stride), rhs = E streams -> PSUM [2, 256] = (denom, numer) rows per
    pair; every matmul start=True/stop=True (no PSUM accumulation -> no
    whole-bank has_written hazards); 4 pairs per result bank.
  - VectorE flushes banks to SBUF; a DRAM scratch bounce re-lays 32 pairs
    into a dense [32, 1024] tile (one writer); VectorE adds the two ktile
    partials, reciprocal_approx_fast + multiply + affine epilogue;
    one contiguous DMA per block to the n-major output.

Weight scalars are baked into the module as immediates (compiled per
weight set, cached) to avoid TensorScalarPtr sync-wait limits.
"""

from contextlib import ExitStack

import numpy as np

import concourse.bacc as bacc
import concourse.bass as bass
import concourse.tile as tile
from concourse import mybir
from concourse.bass_utils import run_bass_kernel_spmd

FP = mybir.dt.float32
AF = mybir.ActivationFunctionType
ALU = mybir.AluOpType

L = 32768          # sequence length
N = 32             # batch
BS = 256           # block size
NB = L // BS       # 128 blocks
NCORES = 8
BPC = NB // NCORES  # 16 blocks per core
LS = BPC * BS       # 4096 rows per core shard

GROUP = 3           # pairs per exp staging group (3 PSUM banks)
PAIRS = BPC * N     # 512 pairs per core
F32R = mybir.dt.float32r
BF16 = mybir.dt.bfloat16
F32R_OUTER = True   # full-rate relaxed-precision fp32 matmul for scores
EDT = BF16          # E dtype for the reductions (rounding cancels in ratio)


def build_kernel_module(sc, reps: int = 1) -> bass.Bass:
    """sc: dict of python-float weight scalars baked as immediates.

    reps > 1 wraps the whole body in a device-side For_i loop — used only
    for wall-clock benchmarking (amplifies device time over dispatch noise).
    """
    nc = bacc.Bacc("TRN2", target_bir_lowering=False, debug=False, num_devices=NCORES)
    # qkt[4n+c, :] = [qT[n, 1024c:1024(c+1)] | kT[n, 1024c:1024(c+1)]]
    qkt = nc.declare_dram_parameter("qkt", [128, 2048], FP, isOutput=False)
    v = nc.declare_dram_parameter("v", [LS, N], FP, isOutput=False)
    out_t = nc.declare_dram_parameter("out_t", [N, LS], FP, isOutput=True)

    with tile.TileContext(nc) as tc:
        with ExitStack() as ctx:
            if reps == 1:
                _emit(ctx, tc, qkt, v, out_t, sc)
            else:
                with tc.For_i(0, reps, 1):
                    _emit(ctx, tc, qkt, v, out_t, sc)
    nc.compile()
    return nc


def _emit(ctx, tc, qkt, v, out_t, sc):
    nc = tc.nc

    rows = ctx.enter_context(tc.tile_pool(name="rows", bufs=1))
    stage = ctx.enter_context(tc.tile_pool(name="stage", bufs=2))
    vpool = ctx.enter_context(tc.tile_pool(name="vpool", bufs=1))
    epool = ctx.enter_context(tc.tile_pool(name="epool", bufs=3))
    dpool = ctx.enter_context(tc.tile_pool(name="dpool", bufs=2))
    ps_stage = ctx.enter_context(tc.tile_pool(name="ps_stage", bufs=2, space="PSUM"))
    ps_res = ctx.enter_context(tc.tile_pool(name="ps_res", bufs=2, space="PSUM"))
    drs = ctx.enter_context(tc.tile_pool(name="drs", bufs=2, space="DRAM"))

    # --- prep ------------------------------------------------------------------
    QKDT = F32R if F32R_OUTER else FP
    qk4 = rows.tile([128, 2048], QKDT, name="qk4", tag="qk4")
    nc.sync.dma_start(out=qk4[:].bitcast(FP), in_=qkt[:])
    nc.vector.tensor_scalar(
        out=qk4[:, 0:1024], in0=qk4[:, 0:1024].bitcast(FP),
        scalar1=sc["wq"], scalar2=sc["bq"], op0=ALU.mult, op1=ALU.add,
    )
    nc.vector.tensor_scalar(
        out=qk4[:, 1024:2048], in0=qk4[:, 1024:2048].bitcast(FP),
        scalar1=sc["wk"], scalar2=sc["bk"], op0=ALU.mult, op1=ALU.add,
    )

    # [ones, v] tiles: col 0 = 1.0 (memset once); cols 1..64 = raw V of the
    # block, [t, n] order.  Two fixed tiles used alternately per block.
    vcombs = []
    for name in ("vcA", "vcB"):
        vc = vpool.tile([128, 2, N, 3], EDT, name=name, tag=name)
        nc.vector.memset(vc[:], 1.0)
        vcombs.append(vc)

    def load_vcomb(b):
        # DMA raw V, then split into bf16 hi+lo columns (exact to ~2^-16).
        vc = vcombs[b % 2]
        vch = vpool.tile([128, 2, N], FP, name="vch", tag="vch", bufs=2)
        hi32 = vpool.tile([128, 2, N], FP, name="hi32", tag="hi32", bufs=2)
        nc.sync.dma_start(
            out=vch[:],
            in_=v[b * BS:(b + 1) * BS, :].rearrange("(t p) n -> p t n", p=128),
        )
        vc4 = vc[:]
        nc.vector.tensor_copy(vc4[:, :, :, 1], vch[:])
        nc.vector.tensor_copy(hi32[:], vc4[:, :, :, 1])
        nc.vector.tensor_sub(vc4[:, :, :, 2], vch[:], hi32[:])
        return vc

    # --- per-half-block q/k row staging (to partition 0) -----------------------
    def stage_rows(b, h):
        # row n (16h <= n < 16h+16): q at [0, (2(n-16h))*256:...],
        #                            k at [0, (2(n-16h)+1)*256:...]
        qks = stage.tile([1, 16 * 2 * BS], QKDT, name="qks", tag="qks")
        qv = qk4[:].rearrange("(n c) (g f) -> n c g f", c=4, g=2)
        cb, cc = b // 4, (b % 4) * BS
        nc.sync.dma_start(out=qks[:], in_=qv[16 * h:16 * (h + 1), cb, :, cc:cc + BS])
        return qks

    # --- main loop --------------------------------------------------------------
    vcur = [None]
    res_state = {"tile": None, "count": 0, "nflush": 0, "rs": None, "first_g": 0}

    def emit_reduces(pend):
        e, members = pend
        for (s, b, n, vc) in members:
            g = b * N + n
            r = res_state["count"]
            if r == 0:
                res_state["tile"] = ps_res.tile([128, 512], FP, name="res", tag="res")
                if res_state["nflush"] == 0:
                    res_state["rs"] = dpool.tile([128, 4096], FP, name="rs", tag="rs")
                    res_state["first_g"] = g
            jj = r
            for t in (0, 1):
                nc.tensor.matmul(
                    res_state["tile"][32 * jj:32 * jj + 3, t * 256:(t + 1) * 256],
                    lhsT=vc[:][:, t, n, :],
                    rhs=e[:][:, s * 512 + t * 256: s * 512 + (t + 1) * 256],
                    start=True, stop=True,
                    tile_position=(0, 32 * jj),
                )
            res_state["count"] += 1
            if res_state["count"] == 4:
                m = res_state["nflush"]
                nc.vector.tensor_copy(
                    res_state["rs"][:, m * 512:(m + 1) * 512], res_state["tile"][:]
                )
                res_state["count"] = 0
                res_state["tile"] = None
                res_state["nflush"] += 1
                if res_state["nflush"] == 8:
                    division_batch()

    def division_batch():
        b0 = res_state["first_g"] // N
        rs = res_state["rs"]
        # rows {32j+r} of rs -> DRAM scratch already in dense layout:
        # scr[4m+j, r*512 + tq] ; then scratch -> dn is a contiguous copy.
        scr = drs.tile([N, 1536], FP, name="scr", tag="scr")
        rsv = rs[:].rearrange("(j p2) (m tq) -> j p2 m tq", j=4, m=8)
        sw = scr[:].rearrange("(m j) (r tq) -> j m r tq", m=8, r=3)
        for r in (0, 1, 2):
            nc.sync.dma_start(out=sw[:, :, r, :], in_=rsv[:, r, :, :])
        # scratch -> dense [32, 1536]: partition 4m+j (= local pair n), free (r,t,q)
        dn = dpool.tile([N, 1536], FP, name="dn", tag="dn")
        nc.sync.dma_start(out=dn[:], in_=scr[:])
        dnv = dn[:].rearrange("p (r t q) -> p r t q", r=3, t=2)
        den = dpool.tile([N, BS], FP, name="den", tag="den")
        num = dpool.tile([N, BS], FP, name="num", tag="num")
        nc.vector.tensor_add(den[:], dnv[:, 0, 0, :], dnv[:, 0, 1, :])
        nc.vector.tensor_add(num[:], dnv[:, 1, 0, :], dnv[:, 1, 1, :])
        nc.vector.tensor_add(num[:], num[:], dnv[:, 2, 0, :])
        nc.vector.tensor_add(num[:], num[:], dnv[:, 2, 1, :])
        nc.vector.reciprocal_approx_fast(out=den[:], in_=den[:])
        ov = dpool.tile([N, BS], FP, name="ov", tag="ov")
        nc.vector.tensor_mul(ov[:], num[:], den[:])
        nc.vector.tensor_scalar(
            out=ov[:], in0=ov[:], scalar1=sc["wvo"], scalar2=sc["bvo"] + sc["bo"],
            op0=ALU.mult, op1=ALU.add,
        )
        nc.sync.dma_start(out=out_t[:, b0 * BS:(b0 + 1) * BS], in_=ov[:])
        res_state["nflush"] = 0
        res_state["rs"] = None

    pending = None
    cur_stage = None
    cur_rows = None
    members = []
    for g in range(PAIRS):
        b, n = divmod(g, N)
        if n == 0:
            vcur[0] = load_vcomb(b)
        if n % 16 == 0:
            cur_rows = stage_rows(b, n // 16)
        qks = cur_rows
        nn = n % 16
        s = g % GROUP
        if s == 0:
            cur_stage = ps_stage.tile([128, GROUP * 512], FP, name="st", tag="st")
            members = []
        for t in (0, 1):
            lhsT = qks[:][0:1, (2 * nn + 1) * BS + t * 128: (2 * nn + 1) * BS + (t + 1) * 128]
            rhs = qks[:][0:1, (2 * nn) * BS: (2 * nn + 1) * BS]
            nc.tensor.matmul(
                cur_stage[:, s * 512 + t * 256: s * 512 + (t + 1) * 256],
                lhsT=lhsT, rhs=rhs,
                start=True, stop=True,
                tile_position=(0, 0),
            )
        members.append((s, b, n, vcur[0]))
        if s == GROUP - 1 or g == PAIRS - 1:
            e = epool.tile([128, GROUP * 512], EDT, name="e", tag="e")
            width = len(members) * 512
            nc.scalar.activation(e[:][:, 0:width], cur_stage[:][:, 0:width], AF.Exp)
            if pending is not None:
                emit_reduces(pending)
            pending = (e, members)
    emit_reduces(pending)
    assert res_state["count"] == 0 and res_state["nflush"] == 0, (
        "pair count must be a multiple of 32 (one block per division batch)"
    )


_CACHE: dict = {}


def _get_nc(sc, reps: int = 1) -> bass.Bass:
    key = (tuple(sorted(sc.items())), reps)
    if key not in _CACHE:
        _CACHE[key] = build_kernel_module(sc, reps)
    return _CACHE[key]


def make_in_maps(query, key, value, in_proj_w, in_proj_b, out_proj_w, out_proj_b):
    q = np.ascontiguousarray(np.asarray(query, dtype=np.float32).reshape(L, N))
    k = np.ascontiguousarray(np.asarray(key, dtype=np.float32).reshape(L, N))
    vv = np.ascontiguousarray(np.asarray(value, dtype=np.float32).reshape(L, N))
    wq, wk, wv = [float(x) for x in np.asarray(in_proj_w, dtype=np.float32).reshape(3)]
    bq, bk, bv = [float(x) for x in np.asarray(in_proj_b, dtype=np.float32).reshape(3)]
    wo = float(np.asarray(out_proj_w, dtype=np.float32).reshape(1)[0])
    bo = float(np.asarray(out_proj_b, dtype=np.float32).reshape(1)[0])
    sc = {"wq": wq, "bq": bq, "wk": wk, "bk": bk,
          "wvo": float(np.float32(wo) * np.float32(wv)),
          "bvo": float(np.float32(wo) * np.float32(bv)), "bo": bo}
    in_maps = []
    for c in range(NCORES):
        sl = slice(c * LS, (c + 1) * LS)
        qr = np.ascontiguousarray(q[sl].T).reshape(N, 4, LS // 4)
        kr = np.ascontiguousarray(k[sl].T).reshape(N, 4, LS // 4)
        qkt_np = np.concatenate([qr, kr], axis=2).reshape(128, 2048)
        in_maps.append({
            "qkt": np.ascontiguousarray(qkt_np),
            "v": np.ascontiguousarray(vv[sl]),
        })
    return in_maps, sc


def run(in_maps, sc, **kwargs):
    return run_bass_kernel_spmd(_get_nc(sc), in_maps, list(range(NCORES)), **kwargs)


def assemble(results) -> np.ndarray:
    outs = [np.asarray(results[c]["out_t"], dtype=np.float32).T for c in range(NCORES)]
    return np.ascontiguousarray(np.concatenate(outs, axis=0)).reshape(L, N, 1)


def kernel(query, key, value, in_proj_w, in_proj_b, out_proj_w, out_proj_b):
    in_maps, sc = make_in_maps(
        query, key, value, in_proj_w, in_proj_b, out_proj_w, out_proj_b
    )
    res = run(in_maps, sc)
    return assemble(res.results)



# revision 2
# speedup vs baseline: 1.0974x; 1.0974x over previous
"""Trainium2 Bass kernel for nn_BlockCrossAttn (block-diagonal attention, E=H=1).

Math per (block b, batch n) pair (256-long vectors q', k', v_eff of the block):
    q' = wq*Q + bq ; k' = wk*K + bk ; v_eff = wo*(wv*V + bv) + bo
    soft[q,k] = softmax_k(q'[q] * k'[k])
    out[q] = sum_k soft[q,k] * v_eff[k]
(The V/out affine folds entirely into v_eff because softmax weights sum
to 1.)  No max-subtraction: |scores| <= ~27 worst case, exp is safe in fp32.

Sharding: 128 blocks of 256 rows; 16 blocks per core across 8 cores
(fully independent, no collectives).

All numeric prep happens HOST-side in make_in_maps (affine projections,
bf16 hi/lo splits, staging layout); the device module is weight- and
data-independent and is compiled exactly once.

Per-core device pipeline (512 pairs):
  - Scores via ONE bf16 matmul per (pair, k-half): contraction dim 3 with
    lhsT = [khi; klo; khi], rhs = [qhi; qhi; qlo], so
    S = khi*qhi + klo*qhi + khi*qlo = k*q exact to ~2^-18.  Streams at
    1 cycle/column (vs 2-4 for fp32/f32r), PSUM out, start/stop=True.
  - ScalarE exp over [128, 1536] PSUM spans -> E (bf16) in SBUF.
  - PE reduction matmuls: lhsT = [ones, vhi, vlo] (v_eff split), rhs = E
    -> PSUM [3, 256] rows (den, num_hi, num_lo) per (pair, t);
    col-group packed 4 pairs per result bank via tile_position=(0, 32j).
  - VectorE flushes banks to SBUF; a DRAM scratch bounce re-lays 32 pairs
    into a dense [32, 1536] tile; VectorE combines partials,
    reciprocal_approx_fast + multiply; one contiguous DMA per block to the
    n-major output.
"""

from contextlib import ExitStack

import numpy as np
import ml_dtypes

import concourse.bacc as bacc
import concourse.bass as bass
import concourse.tile as tile
from concourse import mybir
from concourse.bass_utils import run_bass_kernel_spmd

FP = mybir.dt.float32
BF16 = mybir.dt.bfloat16
AF = mybir.ActivationFunctionType
ALU = mybir.AluOpType

L = 32768          # sequence length
N = 32             # batch
BS = 256           # block size
NB = L // BS       # 128 blocks
NCORES = 8
BPC = NB // NCORES  # 16 blocks per core
LS = BPC * BS       # 4096 rows per core shard

GROUP = 3           # pairs per exp staging group (3 PSUM banks)
PAIRS = BPC * N     # 512 pairs per core
SPG = 16            # pairs per q/k stage DMA
PW = 512            # bf16 cols per pair in the qk stage (256 rhs + 2*128 lhsT)

BF_NP = ml_dtypes.bfloat16


def build_kernel_module(reps: int = 1) -> bass.Bass:
    """reps > 1 wraps the body in a device-side For_i loop (benchmarking)."""
    nc = bacc.Bacc("TRN2", target_bir_lowering=False, debug=False, num_devices=NCORES)
    # qks[r, s*SPG*PW + j*PW + c]: pair g = s*SPG+j; per pair slot:
    #   cols 0:256   rows (qhi, qhi, qlo)   -> rhs [3, 256]
    #   cols 256:384 rows (khi0, klo0, khi0) -> lhsT t=0 [3, 128]
    #   cols 384:512 rows (khi1, klo1, khi1) -> lhsT t=1 [3, 128]
    qkst = nc.declare_dram_parameter("qkst", [3, PAIRS * PW], BF16, isOutput=False)
    # vtab[p, b*192 + t*96 + n*3 + c]: c = (1.0, vhi, vlo) of v_eff[b*BS+t*128+p, n]
    vtab = nc.declare_dram_parameter("vtab", [128, BPC * 2 * N * 3], BF16, isOutput=False)
    out_t = nc.declare_dram_parameter("out_t", [N, LS], FP, isOutput=True)

    with tile.TileContext(nc) as tc:
        with ExitStack() as ctx:
            if reps == 1:
                _emit(ctx, tc, qkst, vtab, out_t)
            else:
                with tc.For_i(0, reps, 1):
                    _emit(ctx, tc, qkst, vtab, out_t)
    nc.compile()
    return nc


def _emit(ctx, tc, qkst, vtab, out_t):
    nc = tc.nc

    stage = ctx.enter_context(tc.tile_pool(name="stage", bufs=3))
    vpool = ctx.enter_context(tc.tile_pool(name="vpool", bufs=2))
    epool = ctx.enter_context(tc.tile_pool(name="epool", bufs=3))
    dpool = ctx.enter_context(tc.tile_pool(name="dpool", bufs=2))
    ps_stage = ctx.enter_context(tc.tile_pool(name="ps_stage", bufs=2, space="PSUM"))
    ps_res = ctx.enter_context(tc.tile_pool(name="ps_res", bufs=2, space="PSUM"))
    drs = ctx.enter_context(tc.tile_pool(name="drs", bufs=2, space="DRAM"))

    def load_stage(s):
        qs = stage.tile([3, SPG * PW], BF16, name="qs", tag="qs")
        nc.sync.dma_start(out=qs[:], in_=qkst[:, s * SPG * PW:(s + 1) * SPG * PW])
        return qs

    def load_vtile(b):
        vt = vpool.tile([128, 2, N, 3], BF16, name="vt", tag="vt")
        nc.sync.dma_start(out=vt[:], in_=vtab[:, b * (2 * N * 3):(b + 1) * (2 * N * 3)])
        return vt

    # --- main loop --------------------------------------------------------------
    vcur = [None]
    res_state = {"tile": None, "count": 0, "nflush": 0, "rs": None, "first_g": 0}

    def emit_reduces(pend):
        e, members = pend
        for (s, b, n, vc) in members:
            g = b * N + n
            r = res_state["count"]
            if r == 0:
                res_state["tile"] = ps_res.tile([128, 512], FP, name="res", tag="res")
                if res_state["nflush"] == 0:
                    res_state["rs"] = dpool.tile([128, 4096], FP, name="rs", tag="rs")
                    res_state["first_g"] = g
            jj = r
            for t in (0, 1):
                nc.tensor.matmul(
                    res_state["tile"][32 * jj:32 * jj + 3, t * 256:(t + 1) * 256],
                    lhsT=vc[:][:, t, n, :],
                    rhs=e[:][:, s * 512 + t * 256: s * 512 + (t + 1) * 256],
                    start=True, stop=True,
                    tile_position=(0, 32 * jj),
                )
            res_state["count"] += 1
            if res_state["count"] == 4:
                m = res_state["nflush"]
                nc.vector.tensor_copy(
                    res_state["rs"][:, m * 512:(m + 1) * 512], res_state["tile"][:]
                )
                res_state["count"] = 0
                res_state["tile"] = None
                res_state["nflush"] += 1
                if res_state["nflush"] == 8:
                    division_batch()

    def division_batch():
        b0 = res_state["first_g"] // N
        rs = res_state["rs"]
        # rows {32j+r} of rs -> DRAM scratch already in dense layout:
        # scr[4m+j, r*512 + tq] ; then scratch -> dn is a contiguous copy.
        scr = drs.tile([N, 1536], FP, name="scr", tag="scr")
        rsv = rs[:].rearrange("(j p2) (m tq) -> j p2 m tq", j=4, m=8)
        sw = scr[:].rearrange("(m j) (r tq) -> j m r tq", m=8, r=3)
        for r in (0, 1, 2):
            nc.sync.dma_start(out=sw[:, :, r, :], in_=rsv[:, r, :, :])
        # scratch -> dense [32, 1536]: partition 4m+j (= local pair n), free (r,t,q)
        dn = dpool.tile([N, 1536], FP, name="dn", tag="dn")
        nc.sync.dma_start(out=dn[:], in_=scr[:])
        dnv = dn[:].rearrange("p (r t q) -> p r t q", r=3, t=2)
        den = dpool.tile([N, BS], FP, name="den", tag="den")
        num = dpool.tile([N, BS], FP, name="num", tag="num")
        nc.vector.tensor_add(den[:], dnv[:, 0, 0, :], dnv[:, 0, 1, :])
        nc.vector.tensor_add(num[:], dnv[:, 1, 0, :], dnv[:, 1, 1, :])
        nc.vector.tensor_add(num[:], num[:], dnv[:, 2, 0, :])
        nc.vector.tensor_add(num[:], num[:], dnv[:, 2, 1, :])
        nc.vector.reciprocal_approx_fast(out=den[:], in_=den[:])
        ov = dpool.tile([N, BS], FP, name="ov", tag="ov")
        nc.vector.tensor_mul(ov[:], num[:], den[:])
        nc.sync.dma_start(out=out_t[:, b0 * BS:(b0 + 1) * BS], in_=ov[:])
        res_state["nflush"] = 0
        res_state["rs"] = None

    pending = None
    cur_stage = None
    cur_qs = None
    members = []
    for g in range(PAIRS):
        b, n = divmod(g, N)
        if n == 0:
            vcur[0] = load_vtile(b)
        if g % SPG == 0:
            cur_qs = load_stage(g // SPG)
        j = g % SPG
        s = g % GROUP
        if s == 0:
            cur_stage = ps_stage.tile([128, GROUP * 512], FP, name="st", tag="st")
            members = []
        qsv = cur_qs[:]
        for t in (0, 1):
            nc.tensor.matmul(
                cur_stage[:, s * 512 + t * 256: s * 512 + (t + 1) * 256],
                lhsT=qsv[:, j * PW + 256 + t * 128: j * PW + 256 + (t + 1) * 128],
                rhs=qsv[:, j * PW: j * PW + 256],
                start=True, stop=True,
                tile_position=(0, 0),
            )
        members.append((s, b, n, vcur[0]))
        if s == GROUP - 1 or g == PAIRS - 1:
            e = epool.tile([128, GROUP * 512], BF16, name="e", tag="e")
            width = len(members) * 512
            nc.scalar.activation(e[:][:, 0:width], cur_stage[:][:, 0:width], AF.Exp)
            if pending is not None:
                emit_reduces(pending)
            pending = (e, members)
    emit_reduces(pending)
    assert res_state["count"] == 0 and res_state["nflush"] == 0, (
        "pair count must be a multiple of 32 (one block per division batch)"
    )


_CACHE: dict = {}


def _get_nc(reps: int = 1) -> bass.Bass:
    if reps not in _CACHE:
        _CACHE[reps] = build_kernel_module(reps)
    return _CACHE[reps]


def _split_bf16(x):
    hi = x.astype(BF_NP)
    lo = (x - hi.astype(np.float32)).astype(BF_NP)
    return hi, lo


def make_in_maps(query, key, value, in_proj_w, in_proj_b, out_proj_w, out_proj_b):
    q = np.asarray(query, dtype=np.float32).reshape(L, N)
    k = np.asarray(key, dtype=np.float32).reshape(L, N)
    vv = np.asarray(value, dtype=np.float32).reshape(L, N)
    wq, wk, wv = [float(x) for x in np.asarray(in_proj_w, dtype=np.float32).reshape(3)]
    bq, bk, bv = [float(x) for x in np.asarray(in_proj_b, dtype=np.float32).reshape(3)]
    wo = float(np.asarray(out_proj_w, dtype=np.float32).reshape(1)[0])
    bo = float(np.asarray(out_proj_b, dtype=np.float32).reshape(1)[0])

    qp = q * np.float32(wq) + np.float32(bq)
    kp = k * np.float32(wk) + np.float32(bk)
    # softmax weights sum to 1 -> the whole v/out affine folds into v:
    veff = (vv * np.float32(wv) + np.float32(bv)) * np.float32(wo) + np.float32(bo)

    qhi, qlo = _split_bf16(qp)
    khi, klo = _split_bf16(kp)
    vhi, vlo = _split_bf16(veff)

    in_maps = []
    for c in range(NCORES):
        sl = slice(c * LS, (c + 1) * LS)
        # [LS, N] core shards -> per-pair vectors; pair g = b*N + n
        def pairs_of(x):
            # -> [PAIRS, BS] (pair-major), x is [LS, N]
            return np.ascontiguousarray(
                x[sl].reshape(BPC, BS, N).transpose(0, 2, 1).reshape(PAIRS, BS)
            )

        qh, ql = pairs_of(qhi), pairs_of(qlo)
        kh, kl = pairs_of(khi), pairs_of(klo)
        qkst = np.empty((3, PAIRS, PW), dtype=BF_NP)
        qkst[0, :, 0:256] = qh
        qkst[1, :, 0:256] = qh
        qkst[2, :, 0:256] = ql
        qkst[0, :, 256:512] = kh
        qkst[1, :, 256:512] = kl
        qkst[2, :, 256:512] = kh
        qkst = np.ascontiguousarray(qkst.reshape(3, PAIRS * PW))

        # vtab[p, (b, t, n, c)] with c = (1, vhi, vlo)
        vt = np.empty((128, BPC, 2, N, 3), dtype=BF_NP)
        vt[:, :, :, :, 0] = np.float32(1.0)
        # vhi[sl] is [LS, N] = [(b t p), n]
        vt[:, :, :, :, 1] = vhi[sl].reshape(BPC, 2, 128, N).transpose(2, 0, 1, 3)
        vt[:, :, :, :, 2] = vlo[sl].reshape(BPC, 2, 128, N).transpose(2, 0, 1, 3)
        vt = np.ascontiguousarray(vt.reshape(128, BPC * 2 * N * 3))

        in_maps.append({"qkst": qkst, "vtab": vt})
    return in_maps, None


def run(in_maps, sc=None, **kwargs):
    return run_bass_kernel_spmd(_get_nc(), in_maps, list(range(NCORES)), **kwargs)


def assemble(results) -> np.ndarray:
    outs = [np.asarray(results[c]["out_t"], dtype=np.float32).T for c in range(NCORES)]
    return np.ascontiguousarray(np.concatenate(outs, axis=0)).reshape(L, N, 1)


def kernel(query, key, value, in_proj_w, in_proj_b, out_proj_w, out_proj_b):
    in_maps, sc = make_in_maps(
        query, key, value, in_proj_w, in_proj_b, out_proj_w, out_proj_b
    )
    res = run(in_maps, sc)
    return assemble(res.results)


# revision 7
# speedup vs baseline: 1.2340x; 1.1245x over previous
"""Trainium2 Bass kernel for nn_BlockCrossAttn (block-diagonal attention, E=H=1).

Math per (block b, batch n) pair (256-long vectors q', k', v_eff of the block):
    q' = wq*Q + bq ; k' = wk*K + bk ; v_eff = wo*(wv*V + bv) + bo
    soft[q,k] = softmax_k(q'[q] * k'[k])
    out[q] = sum_k soft[q,k] * v_eff[k]
(The V/out affine folds entirely into v_eff because softmax weights sum
to 1.)  No max-subtraction: |scores| <= ~27 worst case, exp is safe in fp32.

Sharding: 128 blocks of 256 rows; 16 blocks per core across 8 cores
(fully independent, no collectives).

All numeric prep happens HOST-side in make_in_maps (affine projections,
bf16 hi/lo splits, staging layout); the device module is weight- and
data-independent and is compiled exactly once.

Per-core device pipeline (512 pairs):
  - Scores via ONE bf16 matmul per (pair, k-half): contraction dim 3 with
    lhsT = [khi; klo; khi], rhs = [qhi; qhi; qlo], so
    S = khi*qhi + klo*qhi + khi*qlo = k*q exact to ~2^-18.  Streams at
    1 cycle/column (vs 2-4 for fp32/f32r), PSUM out, start/stop=True.
  - ScalarE exp over [128, 1536] PSUM spans -> E (bf16) in SBUF.
  - PE reduction matmuls: lhsT = [ones, vhi, vlo] (v_eff split), rhs = E
    -> PSUM [3, 256] rows (den, num_hi, num_lo) per (pair, t);
    col-group packed 4 pairs per result bank via tile_position=(0, 32j).
  - VectorE flushes banks to SBUF; a DRAM scratch bounce re-lays 32 pairs
    into a dense [32, 1536] tile; VectorE combines partials,
    reciprocal_approx_fast + multiply; one contiguous DMA per block to the
    n-major output.
"""

from contextlib import ExitStack

import numpy as np
import ml_dtypes

import concourse.bacc as bacc
import concourse.bass as bass
import concourse.tile as tile
from concourse import mybir
from concourse.bass_utils import run_bass_kernel_spmd

FP = mybir.dt.float32
BF16 = mybir.dt.bfloat16
AF = mybir.ActivationFunctionType
ALU = mybir.AluOpType

L = 32768          # sequence length
N = 32             # batch
BS = 256           # block size
NB = L // BS       # 128 blocks
NCORES = 8
BPC = NB // NCORES  # 16 blocks per core
LS = BPC * BS       # 4096 rows per core shard

GROUP = 3           # pairs per exp staging group (3 PSUM banks)
PAIRS = BPC * N     # 512 pairs per core
SPG = 16            # pairs per q/k stage DMA
PW = 512            # bf16 cols per pair in the qk stage (256 rhs + 2*128 lhsT)

BF_NP = ml_dtypes.bfloat16


def build_kernel_module(reps: int = 1) -> bass.Bass:
    """reps > 1 wraps the body in a device-side For_i loop (benchmarking)."""
    nc = bacc.Bacc("TRN2", target_bir_lowering=False, debug=False, num_devices=NCORES)
    # Score matmuls are packed 4-at-a-time into the four 32-row PE groups
    # (tile_position=(32a, 0)); pair g uses row group a = g % 4, i.e. SBUF
    # partitions 32a..32a+2.  qkst row r = 3a + c maps to partition 32a + c.
    # Per pair slot (cols g*PW ..):
    #   cols 0:256   rows (qhi, qhi, qlo)    -> rhs [3, 256]
    #   cols 256:384 rows (khi0, klo0, khi0) -> lhsT t=0 [3, 128]
    #   cols 384:512 rows (khi1, klo1, khi1) -> lhsT t=1 [3, 128]
    qkst = nc.declare_dram_parameter("qkst", [12, (PAIRS // 4) * PW], BF16, isOutput=False)
    # vtab[p, b*192 + t*96 + n*3 + c]: c = (1.0, vhi, vlo) of v_eff[b*BS+t*128+p, n]
    vtab = nc.declare_dram_parameter("vtab", [128, BPC * 2 * N * 3], BF16, isOutput=False)
    out_t = nc.declare_dram_parameter("out_t", [N, LS], FP, isOutput=True)

    with tile.TileContext(nc) as tc:
        with ExitStack() as ctx:
            if reps == 1:
                _emit(ctx, tc, qkst, vtab, out_t)
            else:
                with tc.For_i(0, reps, 1):
                    _emit(ctx, tc, qkst, vtab, out_t)
    nc.compile()
    return nc


def _emit(ctx, tc, qkst, vtab, out_t):
    nc = tc.nc

    stage = ctx.enter_context(tc.tile_pool(name="stage", bufs=3))
    vpool = ctx.enter_context(tc.tile_pool(name="vpool", bufs=2))
    epool = ctx.enter_context(tc.tile_pool(name="epool", bufs=3))
    dpool = ctx.enter_context(tc.tile_pool(name="dpool", bufs=2))
    ps_stage = ctx.enter_context(tc.tile_pool(name="ps_stage", bufs=2, space="PSUM"))
    ps_res = ctx.enter_context(tc.tile_pool(name="ps_res", bufs=2, space="PSUM"))
    drs = ctx.enter_context(tc.tile_pool(name="drs", bufs=2, space="DRAM"))

    def load_stage(s):
        # SPG pairs = SPG/4 quads; 12 dram rows -> partitions {32a + c}
        qs = stage.tile([128, (SPG // 4) * PW], BF16, name="qs", tag="qs")
        w0 = s * (SPG // 4) * PW
        # NB: one dma per row group — a single strided-partition view write
        # is not reliably ordered against the sliced matmul reads.
        for a in range(4):
            nc.sync.dma_start(
                out=qs[32 * a:32 * a + 3, :],
                in_=qkst[3 * a:3 * a + 3, w0:w0 + (SPG // 4) * PW],
            )
        return qs

    def load_vtile(b):
        vt = vpool.tile([128, 2, N, 3], BF16, name="vt", tag="vt")
        nc.sync.dma_start(out=vt[:], in_=vtab[:, b * (2 * N * 3):(b + 1) * (2 * N * 3)])
        return vt

    # --- main loop --------------------------------------------------------------
    vcur = [None]
    res_state = {"tile": None, "count": 0, "nflush": 0, "rs": None, "first_g": 0}

    def emit_reduces(pend):
        e, members = pend
        for (s, b, n, vc) in members:
            g = b * N + n
            r = res_state["count"]
            if r == 0:
                res_state["tile"] = ps_res.tile([128, 512], FP, name="res", tag="res")
                if res_state["nflush"] == 0:
                    res_state["rs"] = dpool.tile([128, 4096], FP, name="rs", tag="rs")
                    res_state["first_g"] = g
            jj = r
            for t in (0, 1):
                nc.tensor.matmul(
                    res_state["tile"][32 * jj:32 * jj + 3, t * 256:(t + 1) * 256],
                    lhsT=vc[:][:, t, n, :],
                    rhs=e[:][:, s * 512 + t * 256: s * 512 + (t + 1) * 256],
                    start=True, stop=True,
                    tile_position=(0, 32 * jj),
                )
            res_state["count"] += 1
            if res_state["count"] == 4:
                m = res_state["nflush"]
                nc.vector.tensor_copy(
                    res_state["rs"][:, m * 512:(m + 1) * 512], res_state["tile"][:]
                )
                res_state["count"] = 0
                res_state["tile"] = None
                res_state["nflush"] += 1
                if res_state["nflush"] == 8:
                    division_batch()

    def division_batch():
        b0 = res_state["first_g"] // N
        rs = res_state["rs"]
        # rows {32j+r} of rs -> DRAM scratch already in dense layout:
        # scr[4m+j, r*512 + tq] ; then scratch -> dn is a contiguous copy.
        scr = drs.tile([N, 1536], FP, name="scr", tag="scr")
        rsv = rs[:].rearrange("(j p2) (m tq) -> j p2 m tq", j=4, m=8)
        sw = scr[:].rearrange("(m j) (r tq) -> j m r tq", m=8, r=3)
        for r in (0, 1, 2):
            nc.sync.dma_start(out=sw[:, :, r, :], in_=rsv[:, r, :, :])
        # scratch -> dense [32, 1536]: partition 4m+j (= local pair n), free (r,t,q)
        dn = dpool.tile([N, 1536], FP, name="dn", tag="dn")
        nc.sync.dma_start(out=dn[:], in_=scr[:])
        dnv = dn[:].rearrange("p (r t q) -> p r t q", r=3, t=2)
        den = dpool.tile([N, BS], FP, name="den", tag="den")
        num = dpool.tile([N, BS], FP, name="num", tag="num")
        nc.vector.tensor_add(den[:], dnv[:, 0, 0, :], dnv[:, 0, 1, :])
        nc.vector.tensor_add(num[:], dnv[:, 1, 0, :], dnv[:, 1, 1, :])
        nc.vector.tensor_add(num[:], num[:], dnv[:, 2, 0, :])
        nc.vector.tensor_add(num[:], num[:], dnv[:, 2, 1, :])
        nc.vector.reciprocal_approx_fast(out=den[:], in_=den[:])
        ov = dpool.tile([N, BS], FP, name="ov", tag="ov")
        nc.vector.tensor_mul(ov[:], num[:], den[:])
        nc.sync.dma_start(out=out_t[:, b0 * BS:(b0 + 1) * BS], in_=ov[:])
        res_state["nflush"] = 0
        res_state["rs"] = None

    pending = None
    cur_qs = None
    gstate = {}   # act-group G -> {"tile": psum tile, "members": [...]}
    next_act = [0]

    def group_of(G):
        if G not in gstate:
            gstate[G] = {
                "tile": ps_stage.tile([128, GROUP * 512], FP, name="st", tag="st"),
                "members": [],
            }
        return gstate[G]

    for w in range(PAIRS // 4):
        g0 = 4 * w
        b = g0 // N
        if g0 % N == 0:
            vcur[0] = load_vtile(b)
        if g0 % SPG == 0:
            cur_qs = load_stage(g0 // SPG)
        wi = w % (SPG // 4)
        qsv = cur_qs[:]
        # one quad: pairs g0..g0+3 at PE row groups 0..3, co-streaming
        for t in (0, 1):
            for a in range(4):
                g = g0 + a
                G, s = divmod(g, GROUP)
                gs = group_of(G)
                if t == 0:
                    gs["members"].append((s, g // N, g % N, vcur[0]))
                nc.tensor.matmul(
                    gs["tile"][:, s * 512 + t * 256: s * 512 + (t + 1) * 256],
                    lhsT=qsv[32 * a:32 * a + 3,
                             wi * PW + 256 + t * 128: wi * PW + 256 + (t + 1) * 128],
                    rhs=qsv[32 * a:32 * a + 3, wi * PW: wi * PW + 256],
                    start=True, stop=True,
                    tile_position=(32 * a, 0),
                )
        # fire exp for every act-group fully covered by pairs <= g0+3
        while next_act[0] * GROUP + GROUP - 1 <= g0 + 3 or (
            g0 + 3 == PAIRS - 1 and next_act[0] in gstate
        ):
            G = next_act[0]
            gs = gstate.pop(G)
            e = epool.tile([128, GROUP * 512], BF16, name="e", tag="e")
            width = len(gs["members"]) * 512
            nc.scalar.activation(e[:][:, 0:width], gs["tile"][:][:, 0:width], AF.Exp)
            if pending is not None:
                emit_reduces(pending)
            pending = (e, gs["members"])
            next_act[0] += 1
    emit_reduces(pending)
    assert res_state["count"] == 0 and res_state["nflush"] == 0, (
        "pair count must be a multiple of 32 (one block per division batch)"
    )


_CACHE: dict = {}


def _get_nc(reps: int = 1) -> bass.Bass:
    if reps not in _CACHE:
        _CACHE[reps] = build_kernel_module(reps)
    return _CACHE[reps]


def _split_bf16(x):
    hi = x.astype(BF_NP)
    lo = (x - hi.astype(np.float32)).astype(BF_NP)
    return hi, lo


def make_in_maps(query, key, value, in_proj_w, in_proj_b, out_proj_w, out_proj_b):
    q = np.asarray(query, dtype=np.float32).reshape(L, N)
    k = np.asarray(key, dtype=np.float32).reshape(L, N)
    vv = np.asarray(value, dtype=np.float32).reshape(L, N)
    wq, wk, wv = [float(x) for x in np.asarray(in_proj_w, dtype=np.float32).reshape(3)]
    bq, bk, bv = [float(x) for x in np.asarray(in_proj_b, dtype=np.float32).reshape(3)]
    wo = float(np.asarray(out_proj_w, dtype=np.float32).reshape(1)[0])
    bo = float(np.asarray(out_proj_b, dtype=np.float32).reshape(1)[0])

    qp = q * np.float32(wq) + np.float32(bq)
    kp = k * np.float32(wk) + np.float32(bk)
    # softmax weights sum to 1 -> the whole v/out affine folds into v:
    veff = (vv * np.float32(wv) + np.float32(bv)) * np.float32(wo) + np.float32(bo)

    qhi, qlo = _split_bf16(qp)
    khi, klo = _split_bf16(kp)
    vhi, vlo = _split_bf16(veff)

    in_maps = []
    for c in range(NCORES):
        sl = slice(c * LS, (c + 1) * LS)
        # [LS, N] core shards -> per-pair vectors; pair g = b*N + n
        def pairs_of(x):
            # -> [PAIRS, BS] (pair-major), x is [LS, N]
            return np.ascontiguousarray(
                x[sl].reshape(BPC, BS, N).transpose(0, 2, 1).reshape(PAIRS, BS)
            )

        qh, ql = pairs_of(qhi), pairs_of(qlo)
        kh, kl = pairs_of(khi), pairs_of(klo)
        NQ = PAIRS // 4
        # row 3a+c <-> SBUF partition 32a+c; quad w holds pairs 4w+a
        qkst = np.empty((4, 3, NQ, PW), dtype=BF_NP)
        for a in range(4):
            sel = slice(a, PAIRS, 4)          # pairs 4w+a
            qkst[a, 0, :, 0:256] = qh[sel]
            qkst[a, 1, :, 0:256] = qh[sel]
            qkst[a, 2, :, 0:256] = ql[sel]
            qkst[a, 0, :, 256:512] = kh[sel]
            qkst[a, 1, :, 256:512] = kl[sel]
            qkst[a, 2, :, 256:512] = kh[sel]
        qkst = np.ascontiguousarray(qkst.reshape(12, NQ * PW))

        # vtab[p, (b, t, n, c)] with c = (1, vhi, vlo)
        vt = np.empty((128, BPC, 2, N, 3), dtype=BF_NP)
        vt[:, :, :, :, 0] = np.float32(1.0)
        # vhi[sl] is [LS, N] = [(b t p), n]
        vt[:, :, :, :, 1] = vhi[sl].reshape(BPC, 2, 128, N).transpose(2, 0, 1, 3)
        vt[:, :, :, :, 2] = vlo[sl].reshape(BPC, 2, 128, N).transpose(2, 0, 1, 3)
        vt = np.ascontiguousarray(vt.reshape(128, BPC * 2 * N * 3))

        in_maps.append({"qkst": qkst, "vtab": vt})
    return in_maps, None


def run(in_maps, sc=None, **kwargs):
    return run_bass_kernel_spmd(_get_nc(), in_maps, list(range(NCORES)), **kwargs)


def assemble(results) -> np.ndarray:
    outs = [np.asarray(results[c]["out_t"], dtype=np.float32).T for c in range(NCORES)]
    return np.ascontiguousarray(np.concatenate(outs, axis=0)).reshape(L, N, 1)


def kernel(query, key, value, in_proj_w, in_proj_b, out_proj_w, out_proj_b):
    in_maps, sc = make_in_maps(
        query, key, value, in_proj_w, in_proj_b, out_proj_w, out_proj_b
    )
    res = run(in_maps, sc)
    return assemble(res.results)


# revision 12
# speedup vs baseline: 1.3661x; 1.1070x over previous
"""Trainium2 Bass kernel for nn_BlockCrossAttn (block-diagonal attention, E=H=1).

Math per (block b, batch n) pair (256-long vectors q', k', v_eff of the block):
    q' = wq*Q + bq ; k' = wk*K + bk ; v_eff = wo*(wv*V + bv) + bo
    soft[q,k] = softmax_k(q'[q] * k'[k])
    out[q] = sum_k soft[q,k] * v_eff[k]
(The V/out affine folds entirely into v_eff because softmax weights sum
to 1.)  No max-subtraction: |scores| <= ~27 worst case, exp is safe in fp32.

Sharding: 128 blocks of 256 rows; 16 blocks per core across 8 cores
(fully independent, no collectives).

All numeric prep happens HOST-side in make_in_maps (affine projections,
bf16 hi/lo splits, staging layout); the device module is weight- and
data-independent and is compiled exactly once.

Per-core device pipeline (512 pairs):
  - Scores via ONE bf16 matmul per (pair, k-half): contraction dim 3 with
    lhsT = [khi; klo; khi], rhs = [qhi; qhi; qlo], so
    S = khi*qhi + klo*qhi + khi*qlo = k*q exact to ~2^-18.  Streams at
    1 cycle/column (vs 2-4 for fp32/f32r), PSUM out, start/stop=True.
  - ScalarE exp over [128, 1536] PSUM spans -> E (bf16) in SBUF.
  - PE reduction matmuls: lhsT = [ones, vhi, vlo] (v_eff split), rhs = E
    -> PSUM [3, 256] rows (den, num_hi, num_lo) per (pair, t);
    col-group packed 4 pairs per result bank via tile_position=(0, 32j).
  - VectorE flushes banks to SBUF; a DRAM scratch bounce re-lays 32 pairs
    into a dense [32, 1536] tile; VectorE combines partials,
    reciprocal_approx_fast + multiply; one contiguous DMA per block to the
    n-major output.
"""

from contextlib import ExitStack

import numpy as np
import ml_dtypes

import concourse.bacc as bacc
import concourse.bass as bass
import concourse.tile as tile
from concourse import mybir
from concourse.bass_utils import run_bass_kernel_spmd

FP = mybir.dt.float32
BF16 = mybir.dt.bfloat16
AF = mybir.ActivationFunctionType
ALU = mybir.AluOpType

L = 32768          # sequence length
N = 32             # batch
BS = 256           # block size
NB = L // BS       # 128 blocks
NCORES = 8
BPC = NB // NCORES  # 16 blocks per core
LS = BPC * BS       # 4096 rows per core shard

GROUP = 3           # pairs per exp staging group (3 PSUM banks)
PAIRS = BPC * N     # 512 pairs per core
NGRP = (PAIRS + GROUP - 1) // GROUP  # 171 act groups (last has 2 pairs)
SPG = 4             # groups (= slots) per q/k stage DMA
PW = 512            # bf16 cols per pair in the qk stage (256 rhs + 2*128 lhsT)

BF_NP = ml_dtypes.bfloat16


def build_kernel_module(reps: int = 1) -> bass.Bass:
    """reps > 1 wraps the body in a device-side For_i loop (benchmarking)."""
    nc = bacc.Bacc("TRN2", target_bir_lowering=False, debug=False, num_devices=NCORES)
    # Score matmuls are packed 3-at-a-time into PE row groups 0/32/64
    # (tile_position=(32s, 0)); pair 3G+s uses row group s, i.e. SBUF
    # partitions 32s..32s+2.  qkst row r = 3s + c maps to partition 32s + c;
    # col slot = act-group index G.  Per pair slot (cols G*PW ..):
    #   cols 0:256   rows (qhi, qhi, qlo)    -> rhs [3, 256]
    #   cols 256:384 rows (khi0, klo0, khi0) -> lhsT t=0 [3, 128]
    #   cols 384:512 rows (khi1, klo1, khi1) -> lhsT t=1 [3, 128]
    qkst = nc.declare_dram_parameter("qkst", [9, NGRP * PW], BF16, isOutput=False)
    # vtab[p, b*192 + t*96 + n*3 + c]: c = (1.0, vhi, vlo) of v_eff[b*BS+t*128+p, n]
    vtab = nc.declare_dram_parameter("vtab", [128, BPC * 2 * N * 3], BF16, isOutput=False)
    out_t = nc.declare_dram_parameter("out_t", [N, LS], FP, isOutput=True)

    with tile.TileContext(nc) as tc:
        with ExitStack() as ctx:
            if reps == 1:
                _emit(ctx, tc, qkst, vtab, out_t)
            else:
                with tc.For_i(0, reps, 1):
                    _emit(ctx, tc, qkst, vtab, out_t)
    nc.compile()
    return nc


def _emit(ctx, tc, qkst, vtab, out_t):
    nc = tc.nc

    stage = ctx.enter_context(tc.tile_pool(name="stage", bufs=3))
    vpool = ctx.enter_context(tc.tile_pool(name="vpool", bufs=2))
    epool = ctx.enter_context(tc.tile_pool(name="epool", bufs=3))
    dpool = ctx.enter_context(tc.tile_pool(name="dpool", bufs=2))
    ps_stage = ctx.enter_context(tc.tile_pool(name="ps_stage", bufs=2, space="PSUM"))
    ps_res = ctx.enter_context(tc.tile_pool(name="ps_res", bufs=2, space="PSUM"))
    drs = ctx.enter_context(tc.tile_pool(name="drs", bufs=2, space="DRAM"))

    def load_stage(w):
        # SPG slots (= act groups); 9 dram rows -> partitions {32s + c}
        qs = stage.tile([128, SPG * PW], BF16, name="qs", tag="qs")
        w0 = w * SPG * PW
        width = min(SPG * PW, NGRP * PW - w0)
        # NB: one dma per row group — a single strided-partition view write
        # is not reliably ordered against the sliced matmul reads.
        for s in range(3):
            nc.sync.dma_start(
                out=qs[32 * s:32 * s + 3, 0:width],
                in_=qkst[3 * s:3 * s + 3, w0:w0 + width],
            )
        return qs

    def load_vtile(b):
        vt = vpool.tile([128, 2, N, 3], BF16, name="vt", tag="vt")
        nc.sync.dma_start(out=vt[:], in_=vtab[:, b * (2 * N * 3):(b + 1) * (2 * N * 3)])
        return vt

    # --- main loop --------------------------------------------------------------
    vcur = [None]
    res_state = {"tile": None, "count": 0, "nflush": 0, "rs": None, "first_g": 0}

    def emit_reduces(pend):
        e, members = pend
        for (s, b, n, vc) in members:
            g = b * N + n
            r = res_state["count"]
            if r == 0:
                res_state["tile"] = ps_res.tile([128, 512], FP, name="res", tag="res")
                if res_state["nflush"] == 0:
                    res_state["rs"] = dpool.tile([128, 4096], FP, name="rs", tag="rs")
                    res_state["first_g"] = g
            jj = r
            for t in (0, 1):
                nc.tensor.matmul(
                    res_state["tile"][32 * jj:32 * jj + 3, t * 256:(t + 1) * 256],
                    lhsT=vc[:][:, t, n, :],
                    rhs=e[:][:, s * 512 + t * 256: s * 512 + (t + 1) * 256],
                    start=True, stop=True,
                    tile_position=(0, 32 * jj),
                )
            res_state["count"] += 1
            if res_state["count"] == 4:
                m = res_state["nflush"]
                nc.vector.tensor_copy(
                    res_state["rs"][:, m * 512:(m + 1) * 512], res_state["tile"][:]
                )
                res_state["count"] = 0
                res_state["tile"] = None
                res_state["nflush"] += 1
                if res_state["nflush"] == 8:
                    division_batch()

    def division_batch():
        b0 = res_state["first_g"] // N
        rs = res_state["rs"]
        # rows {32j+r} of rs -> DRAM scratch already in dense layout:
        # scr[4m+j, r*512 + tq] ; then scratch -> dn is a contiguous copy.
        scr = drs.tile([N, 1536], FP, name="scr", tag="scr")
        rsv = rs[:].rearrange("(j p2) (m tq) -> j p2 m tq", j=4, m=8)
        sw = scr[:].rearrange("(m j) (r tq) -> j m r tq", m=8, r=3)
        for r in (0, 1, 2):
            nc.sync.dma_start(out=sw[:, :, r, :], in_=rsv[:, r, :, :])
        # scratch -> dense [32, 1536]: partition 4m+j (= local pair n), free (r,t,q)
        dn = dpool.tile([N, 1536], FP, name="dn", tag="dn")
        nc.sync.dma_start(out=dn[:], in_=scr[:])
        dnv = dn[:].rearrange("p (r t q) -> p r t q", r=3, t=2)
        den = dpool.tile([N, BS], FP, name="den", tag="den")
        num = dpool.tile([N, BS], FP, name="num", tag="num")
        nc.vector.tensor_add(den[:], dnv[:, 0, 0, :], dnv[:, 0, 1, :])
        nc.vector.tensor_add(num[:], dnv[:, 1, 0, :], dnv[:, 1, 1, :])
        nc.vector.tensor_add(num[:], num[:], dnv[:, 2, 0, :])
        nc.vector.tensor_add(num[:], num[:], dnv[:, 2, 1, :])
        nc.vector.reciprocal_approx_fast(out=den[:], in_=den[:])
        ov = dpool.tile([N, BS], FP, name="ov", tag="ov")
        nc.vector.tensor_mul(ov[:], num[:], den[:])
        nc.sync.dma_start(out=out_t[:, b0 * BS:(b0 + 1) * BS], in_=ov[:])
        res_state["nflush"] = 0
        res_state["rs"] = None

    pending = None
    cur_qs = None
    for G in range(NGRP):
        gpairs = [g for g in range(GROUP * G, min(GROUP * (G + 1), PAIRS))]
        if G % SPG == 0:
            cur_qs = load_stage(G // SPG)
        wi = G % SPG
        qsv = cur_qs[:]
        cur_stage = ps_stage.tile([128, GROUP * 512], FP, name="st", tag="st")
        members = []
        for s, g in enumerate(gpairs):
            if g % N == 0:
                vcur[0] = load_vtile(g // N)
            members.append((s, g // N, g % N, vcur[0]))
        # waves of 3 co-streaming row-group-tiled score matmuls (t-outer:
        # a pair's two halves share a row group and serialize; distinct
        # pairs use distinct row groups AND distinct PSUM banks)
        for t in (0, 1):
            for s, g in enumerate(gpairs):
                nc.tensor.matmul(
                    cur_stage[:, s * 512 + t * 256: s * 512 + (t + 1) * 256],
                    lhsT=qsv[32 * s:32 * s + 3,
                             wi * PW + 256 + t * 128: wi * PW + 256 + (t + 1) * 128],
                    rhs=qsv[32 * s:32 * s + 3, wi * PW: wi * PW + 256],
                    start=True, stop=True,
                    tile_position=(32 * s, 0),
                )
        e = epool.tile([128, GROUP * 512], BF16, name="e", tag="e")
        width = len(members) * 512
        nc.scalar.activation(e[:][:, 0:width], cur_stage[:][:, 0:width], AF.Exp)
        if pending is not None:
            emit_reduces(pending)
        pending = (e, members)
    emit_reduces(pending)
    assert res_state["count"] == 0 and res_state["nflush"] == 0, (
        "pair count must be a multiple of 32 (one block per division batch)"
    )


_CACHE: dict = {}


def _get_nc(reps: int = 1) -> bass.Bass:
    if reps not in _CACHE:
        _CACHE[reps] = build_kernel_module(reps)
    return _CACHE[reps]


def _split_bf16(x):
    hi = x.astype(BF_NP)
    lo = (x - hi.astype(np.float32)).astype(BF_NP)
    return hi, lo


def make_in_maps(query, key, value, in_proj_w, in_proj_b, out_proj_w, out_proj_b):
    q = np.asarray(query, dtype=np.float32).reshape(L, N)
    k = np.asarray(key, dtype=np.float32).reshape(L, N)
    vv = np.asarray(value, dtype=np.float32).reshape(L, N)
    wq, wk, wv = [float(x) for x in np.asarray(in_proj_w, dtype=np.float32).reshape(3)]
    bq, bk, bv = [float(x) for x in np.asarray(in_proj_b, dtype=np.float32).reshape(3)]
    wo = float(np.asarray(out_proj_w, dtype=np.float32).reshape(1)[0])
    bo = float(np.asarray(out_proj_b, dtype=np.float32).reshape(1)[0])

    qp = q * np.float32(wq) + np.float32(bq)
    kp = k * np.float32(wk) + np.float32(bk)
    # softmax weights sum to 1 -> the whole v/out affine folds into v:
    veff = (vv * np.float32(wv) + np.float32(bv)) * np.float32(wo) + np.float32(bo)

    qhi, qlo = _split_bf16(qp)
    khi, klo = _split_bf16(kp)
    vhi, vlo = _split_bf16(veff)

    in_maps = []
    for c in range(NCORES):
        sl = slice(c * LS, (c + 1) * LS)
        # [LS, N] core shards -> per-pair vectors; pair g = b*N + n
        def pairs_of(x):
            # -> [PAIRS, BS] (pair-major), x is [LS, N]
            return np.ascontiguousarray(
                x[sl].reshape(BPC, BS, N).transpose(0, 2, 1).reshape(PAIRS, BS)
            )

        qh, ql = pairs_of(qhi), pairs_of(qlo)
        kh, kl = pairs_of(khi), pairs_of(klo)
        # row 3s+c <-> SBUF partition 32s+c; col slot = act group G; pair 3G+s
        qkst = np.zeros((3, 3, NGRP, PW), dtype=BF_NP)
        for s in range(3):
            sel = slice(s, PAIRS, 3)          # pairs 3G+s
            cnt = len(range(PAIRS)[sel])
            qkst[s, 0, :cnt, 0:256] = qh[sel]
            qkst[s, 1, :cnt, 0:256] = qh[sel]
            qkst[s, 2, :cnt, 0:256] = ql[sel]
            qkst[s, 0, :cnt, 256:512] = kh[sel]
            qkst[s, 1, :cnt, 256:512] = kl[sel]
            qkst[s, 2, :cnt, 256:512] = kh[sel]
        qkst = np.ascontiguousarray(qkst.reshape(9, NGRP * PW))

        # vtab[p, (b, t, n, c)] with c = (1, vhi, vlo)
        vt = np.empty((128, BPC, 2, N, 3), dtype=BF_NP)
        vt[:, :, :, :, 0] = np.float32(1.0)
        # vhi[sl] is [LS, N] = [(b t p), n]
        vt[:, :, :, :, 1] = vhi[sl].reshape(BPC, 2, 128, N).transpose(2, 0, 1, 3)
        vt[:, :, :, :, 2] = vlo[sl].reshape(BPC, 2, 128, N).transpose(2, 0, 1, 3)
        vt = np.ascontiguousarray(vt.reshape(128, BPC * 2 * N * 3))

        in_maps.append({"qkst": qkst, "vtab": vt})
    return in_maps, None


def run(in_maps, sc=None, **kwargs):
    return run_bass_kernel_spmd(_get_nc(), in_maps, list(range(NCORES)), **kwargs)


def assemble(results) -> np.ndarray:
    outs = [np.asarray(results[c]["out_t"], dtype=np.float32).T for c in range(NCORES)]
    return np.ascontiguousarray(np.concatenate(outs, axis=0)).reshape(L, N, 1)


def kernel(query, key, value, in_proj_w, in_proj_b, out_proj_w, out_proj_b):
    in_maps, sc = make_in_maps(
        query, key, value, in_proj_w, in_proj_b, out_proj_w, out_proj_b
    )
    res = run(in_maps, sc)
    return assemble(res.results)


# revision 14
# speedup vs baseline: 1.4265x; 1.0442x over previous
"""Trainium2 Bass kernel for nn_BlockCrossAttn (block-diagonal attention, E=H=1).

Math per (block b, batch n) pair (256-long vectors q', k', v_eff of the block):
    q' = wq*Q + bq ; k' = wk*K + bk ; v_eff = wo*(wv*V + bv) + bo
    soft[q,k] = softmax_k(q'[q] * k'[k])
    out[q] = sum_k soft[q,k] * v_eff[k]
(The V/out affine folds entirely into v_eff because softmax weights sum
to 1.)  No max-subtraction: |scores| <= ~27 worst case, exp is safe in fp32.

Sharding: 128 blocks of 256 rows; 16 blocks per core across 8 cores
(fully independent, no collectives).

All numeric prep happens HOST-side in make_in_maps (affine projections,
bf16 hi/lo splits, staging layout); the device module is weight- and
data-independent and is compiled exactly once.

Per-core device pipeline (512 pairs):
  - Scores via ONE bf16 matmul per (pair, k-half): contraction dim 3 with
    lhsT = [khi; klo; khi], rhs = [qhi; qhi; qlo], so
    S = khi*qhi + klo*qhi + khi*qlo = k*q exact to ~2^-18.  Streams at
    1 cycle/column (vs 2-4 for fp32/f32r), PSUM out, start/stop=True.
  - ScalarE exp over [128, 1536] PSUM spans -> E (bf16) in SBUF.
  - PE reduction matmuls: lhsT = [ones, vhi, vlo] (v_eff split), rhs = E
    -> PSUM [3, 256] rows (den, num_hi, num_lo) per (pair, t);
    col-group packed 4 pairs per result bank via tile_position=(0, 32j).
  - VectorE flushes banks to SBUF; a DRAM scratch bounce re-lays 32 pairs
    into a dense [32, 1536] tile; VectorE combines partials,
    reciprocal_approx_fast + multiply; one contiguous DMA per block to the
    n-major output.
"""

from contextlib import ExitStack

import numpy as np
import ml_dtypes

import concourse.bacc as bacc
import concourse.bass as bass
import concourse.tile as tile
from concourse import mybir
from concourse.bass_utils import run_bass_kernel_spmd

FP = mybir.dt.float32
BF16 = mybir.dt.bfloat16
AF = mybir.ActivationFunctionType
ALU = mybir.AluOpType

L = 32768          # sequence length
N = 32             # batch
BS = 256           # block size
NB = L // BS       # 128 blocks
NCORES = 8
BPC = NB // NCORES  # 16 blocks per core
LS = BPC * BS       # 4096 rows per core shard

GROUP = 3           # pairs per exp staging group (3 PSUM banks)
PAIRS = BPC * N     # 512 pairs per core
NGRP = (PAIRS + GROUP - 1) // GROUP  # 171 act groups (last has 2 pairs)
SPG = 4             # groups (= slots) per q/k stage DMA
PW = 512            # bf16 cols per pair in the qk stage (256 rhs + 2*128 lhsT)

BF_NP = ml_dtypes.bfloat16


def build_kernel_module(reps: int = 1) -> bass.Bass:
    """reps > 1 wraps the body in a device-side For_i loop (benchmarking)."""
    nc = bacc.Bacc("TRN2", target_bir_lowering=False, debug=False, num_devices=NCORES)
    # Score matmuls are packed 3-at-a-time into PE row groups 0/32/64
    # (tile_position=(32s, 0)); pair 3G+s uses row group s, i.e. SBUF
    # partitions 32s..32s+2.  qkst row r = 3s + c maps to partition 32s + c;
    # col slot = act-group index G.  Per pair slot (cols G*PW ..):
    #   cols 0:256   rows (qhi, qhi, qlo)    -> rhs [3, 256]
    #   cols 256:384 rows (khi0, klo0, khi0) -> lhsT t=0 [3, 128]
    #   cols 384:512 rows (khi1, klo1, khi1) -> lhsT t=1 [3, 128]
    qkst = nc.declare_dram_parameter("qkst", [9, NGRP * PW], BF16, isOutput=False)
    # vtab[p, b*192 + t*96 + n*3 + c]: c = (1.0, vhi, vlo) of v_eff[b*BS+t*128+p, n]
    vtab = nc.declare_dram_parameter("vtab", [128, BPC * 2 * N * 3], BF16, isOutput=False)
    out_t = nc.declare_dram_parameter("out_t", [N, LS], FP, isOutput=True)

    with tile.TileContext(nc) as tc:
        with ExitStack() as ctx:
            if reps == 1:
                _emit(ctx, tc, qkst, vtab, out_t)
            else:
                with tc.For_i(0, reps, 1):
                    _emit(ctx, tc, qkst, vtab, out_t)
    nc.compile()
    return nc


def _emit(ctx, tc, qkst, vtab, out_t):
    nc = tc.nc

    stage = ctx.enter_context(tc.tile_pool(name="stage", bufs=3))
    vpool = ctx.enter_context(tc.tile_pool(name="vpool", bufs=2))
    epool = ctx.enter_context(tc.tile_pool(name="epool", bufs=3))
    dpool = ctx.enter_context(tc.tile_pool(name="dpool", bufs=2))
    rspool = ctx.enter_context(tc.tile_pool(name="rspool", bufs=3))
    ps_stage = ctx.enter_context(tc.tile_pool(name="ps_stage", bufs=2, space="PSUM"))
    ps_res = ctx.enter_context(tc.tile_pool(name="ps_res", bufs=2, space="PSUM"))
    drs = ctx.enter_context(tc.tile_pool(name="drs", bufs=2, space="DRAM"))

    def load_stage(w):
        # SPG slots (= act groups); 9 dram rows -> partitions {32s + c}
        qs = stage.tile([128, SPG * PW], BF16, name="qs", tag="qs")
        w0 = w * SPG * PW
        width = min(SPG * PW, NGRP * PW - w0)
        # NB: one dma per row group — a single strided-partition view write
        # is not reliably ordered against the sliced matmul reads.
        for s in range(3):
            nc.sync.dma_start(
                out=qs[32 * s:32 * s + 3, 0:width],
                in_=qkst[3 * s:3 * s + 3, w0:w0 + width],
            )
        return qs

    def load_vtile(b):
        vt = vpool.tile([128, 2, N, 3], BF16, name="vt", tag="vt")
        nc.sync.dma_start(out=vt[:], in_=vtab[:, b * (2 * N * 3):(b + 1) * (2 * N * 3)])
        return vt

    # --- main loop --------------------------------------------------------------
    vcur = [None]
    res_state = {"tile": None, "count": 0, "nflush": 0, "rs": None, "first_g": 0}

    def emit_reduces(pend):
        e, members = pend
        for (s, b, n, vc) in members:
            g = b * N + n
            r = res_state["count"]
            if r == 0:
                res_state["tile"] = ps_res.tile([128, 256], FP, name="res", tag="res")
                if res_state["nflush"] == 0:
                    res_state["rs"] = rspool.tile([128, 2048], FP, name="rs", tag="rs")
                    res_state["first_g"] = g
            jj = r
            # the two k-halves accumulate in PSUM: rows = (den, num_hi, num_lo)
            for t in (0, 1):
                nc.tensor.matmul(
                    res_state["tile"][32 * jj:32 * jj + 3, 0:256],
                    lhsT=vc[:][:, t, n, :],
                    rhs=e[:][:, s * 512 + t * 256: s * 512 + (t + 1) * 256],
                    start=(t == 0), stop=(t == 1),
                    tile_position=(0, 32 * jj),
                )
            res_state["count"] += 1
            if res_state["count"] == 4:
                m = res_state["nflush"]
                nc.vector.tensor_copy(
                    res_state["rs"][:, m * 256:(m + 1) * 256], res_state["tile"][:]
                )
                res_state["count"] = 0
                res_state["tile"] = None
                res_state["nflush"] += 1
                if res_state["nflush"] == 8:
                    division_batch()

    def division_batch():
        b0 = res_state["first_g"] // N
        rs = res_state["rs"]
        # rows {32j+r} of rs -> DRAM scratch already in dense layout:
        # scr[4m+j, r*256 + q] ; then scratch -> dn is a contiguous copy.
        scr = drs.tile([N, 768], FP, name="scr", tag="scr")
        rsv = rs[:].rearrange("(j p2) (m q) -> j p2 m q", j=4, m=8)
        sw = scr[:].rearrange("(m j) (r q) -> j m r q", m=8, r=3)
        for r in (0, 1, 2):
            nc.sync.dma_start(out=sw[:, :, r, :], in_=rsv[:, r, :, :])
        # scratch -> dense [32, 768]: partition 4m+j (= local pair n), free (r,q)
        dn = dpool.tile([N, 768], FP, name="dn", tag="dn")
        nc.sync.dma_start(out=dn[:], in_=scr[:])
        dnv = dn[:].rearrange("p (r q) -> p r q", r=3)
        den = dpool.tile([N, BS], FP, name="den", tag="den")
        num = dpool.tile([N, BS], FP, name="num", tag="num")
        nc.vector.tensor_add(num[:], dnv[:, 1, :], dnv[:, 2, :])
        nc.vector.reciprocal_approx_fast(out=den[:], in_=dnv[:, 0, :])
        ov = dpool.tile([N, BS], FP, name="ov", tag="ov")
        nc.vector.tensor_mul(ov[:], num[:], den[:])
        nc.sync.dma_start(out=out_t[:, b0 * BS:(b0 + 1) * BS], in_=ov[:])
        res_state["nflush"] = 0
        res_state["rs"] = None

    pending = None
    cur_qs = None
    for G in range(NGRP):
        gpairs = [g for g in range(GROUP * G, min(GROUP * (G + 1), PAIRS))]
        if G % SPG == 0:
            cur_qs = load_stage(G // SPG)
        wi = G % SPG
        qsv = cur_qs[:]
        cur_stage = ps_stage.tile([128, GROUP * 512], FP, name="st", tag="st")
        members = []
        for s, g in enumerate(gpairs):
            if g % N == 0:
                vcur[0] = load_vtile(g // N)
            members.append((s, g // N, g % N, vcur[0]))
        # waves of 3 co-streaming row-group-tiled score matmuls (t-outer:
        # a pair's two halves share a row group and serialize; distinct
        # pairs use distinct row groups AND distinct PSUM banks)
        for t in (0, 1):
            for s, g in enumerate(gpairs):
                nc.tensor.matmul(
                    cur_stage[:, s * 512 + t * 256: s * 512 + (t + 1) * 256],
                    lhsT=qsv[32 * s:32 * s + 3,
                             wi * PW + 256 + t * 128: wi * PW + 256 + (t + 1) * 128],
                    rhs=qsv[32 * s:32 * s + 3, wi * PW: wi * PW + 256],
                    start=True, stop=True,
                    tile_position=(32 * s, 0),
                )
        e = epool.tile([128, GROUP * 512], BF16, name="e", tag="e")
        width = len(members) * 512
        nc.scalar.activation(e[:][:, 0:width], cur_stage[:][:, 0:width], AF.Exp)
        if pending is not None:
            emit_reduces(pending)
        pending = (e, members)
    emit_reduces(pending)
    assert res_state["count"] == 0 and res_state["nflush"] == 0, (
        "pair count must be a multiple of 32 (one block per division batch)"
    )


_CACHE: dict = {}


def _get_nc(reps: int = 1) -> bass.Bass:
    if reps not in _CACHE:
        _CACHE[reps] = build_kernel_module(reps)
    return _CACHE[reps]


def _split_bf16(x):
    hi = x.astype(BF_NP)
    lo = (x - hi.astype(np.float32)).astype(BF_NP)
    return hi, lo


def make_in_maps(query, key, value, in_proj_w, in_proj_b, out_proj_w, out_proj_b):
    q = np.asarray(query, dtype=np.float32).reshape(L, N)
    k = np.asarray(key, dtype=np.float32).reshape(L, N)
    vv = np.asarray(value, dtype=np.float32).reshape(L, N)
    wq, wk, wv = [float(x) for x in np.asarray(in_proj_w, dtype=np.float32).reshape(3)]
    bq, bk, bv = [float(x) for x in np.asarray(in_proj_b, dtype=np.float32).reshape(3)]
    wo = float(np.asarray(out_proj_w, dtype=np.float32).reshape(1)[0])
    bo = float(np.asarray(out_proj_b, dtype=np.float32).reshape(1)[0])

    qp = q * np.float32(wq) + np.float32(bq)
    kp = k * np.float32(wk) + np.float32(bk)
    # softmax weights sum to 1 -> the whole v/out affine folds into v:
    veff = (vv * np.float32(wv) + np.float32(bv)) * np.float32(wo) + np.float32(bo)

    qhi, qlo = _split_bf16(qp)
    khi, klo = _split_bf16(kp)
    vhi, vlo = _split_bf16(veff)

    in_maps = []
    for c in range(NCORES):
        sl = slice(c * LS, (c + 1) * LS)
        # [LS, N] core shards -> per-pair vectors; pair g = b*N + n
        def pairs_of(x):
            # -> [PAIRS, BS] (pair-major), x is [LS, N]
            return np.ascontiguousarray(
                x[sl].reshape(BPC, BS, N).transpose(0, 2, 1).reshape(PAIRS, BS)
            )

        qh, ql = pairs_of(qhi), pairs_of(qlo)
        kh, kl = pairs_of(khi), pairs_of(klo)
        # row 3s+c <-> SBUF partition 32s+c; col slot = act group G; pair 3G+s
        qkst = np.zeros((3, 3, NGRP, PW), dtype=BF_NP)
        for s in range(3):
            sel = slice(s, PAIRS, 3)          # pairs 3G+s
            cnt = len(range(PAIRS)[sel])
            qkst[s, 0, :cnt, 0:256] = qh[sel]
            qkst[s, 1, :cnt, 0:256] = qh[sel]
            qkst[s, 2, :cnt, 0:256] = ql[sel]
            qkst[s, 0, :cnt, 256:512] = kh[sel]
            qkst[s, 1, :cnt, 256:512] = kl[sel]
            qkst[s, 2, :cnt, 256:512] = kh[sel]
        qkst = np.ascontiguousarray(qkst.reshape(9, NGRP * PW))

        # vtab[p, (b, t, n, c)] with c = (1, vhi, vlo)
        vt = np.empty((128, BPC, 2, N, 3), dtype=BF_NP)
        vt[:, :, :, :, 0] = np.float32(1.0)
        # vhi[sl] is [LS, N] = [(b t p), n]
        vt[:, :, :, :, 1] = vhi[sl].reshape(BPC, 2, 128, N).transpose(2, 0, 1, 3)
        vt[:, :, :, :, 2] = vlo[sl].reshape(BPC, 2, 128, N).transpose(2, 0, 1, 3)
        vt = np.ascontiguousarray(vt.reshape(128, BPC * 2 * N * 3))

        in_maps.append({"qkst": qkst, "vtab": vt})
    return in_maps, None


def run(in_maps, sc=None, **kwargs):
    return run_bass_kernel_spmd(_get_nc(), in_maps, list(range(NCORES)), **kwargs)


def assemble(results) -> np.ndarray:
    outs = [np.asarray(results[c]["out_t"], dtype=np.float32).T for c in range(NCORES)]
    return np.ascontiguousarray(np.concatenate(outs, axis=0)).reshape(L, N, 1)


def kernel(query, key, value, in_proj_w, in_proj_b, out_proj_w, out_proj_b):
    in_maps, sc = make_in_maps(
        query, key, value, in_proj_w, in_proj_b, out_proj_w, out_proj_b
    )
    res = run(in_maps, sc)
    return assemble(res.results)


# revision 15
# speedup vs baseline: 1.5174x; 1.0637x over previous
"""Trainium2 Bass kernel for nn_BlockCrossAttn (block-diagonal attention, E=H=1).

Math per (block b, batch n) pair (256-long vectors q', k', v_eff of the block):
    q' = wq*Q + bq ; k' = wk*K + bk ; v_eff = wo*(wv*V + bv) + bo
    soft[q,k] = softmax_k(q'[q] * k'[k])
    out[q] = sum_k soft[q,k] * v_eff[k]
(The V/out affine folds entirely into v_eff because softmax weights sum
to 1.)  No max-subtraction: |scores| <= ~27 worst case, exp is safe in fp32.

Sharding: 128 blocks of 256 rows; 16 blocks per core across 8 cores
(fully independent, no collectives).

All numeric prep happens HOST-side in make_in_maps (affine projections,
bf16 hi/lo splits, staging layout); the device module is weight- and
data-independent and is compiled exactly once.

Per-core device pipeline (512 pairs):
  - Scores via ONE bf16 matmul per (pair, k-half): contraction dim 3 with
    lhsT = [khi; klo; khi], rhs = [qhi; qhi; qlo], so
    S = khi*qhi + klo*qhi + khi*qlo = k*q exact to ~2^-18.  Streams at
    1 cycle/column (vs 2-4 for fp32/f32r), PSUM out, start/stop=True.
  - ScalarE exp over [128, 1536] PSUM spans -> E (bf16) in SBUF.
  - PE reduction matmuls: lhsT = [ones, vhi, vlo] (v_eff split), rhs = E
    -> PSUM [3, 256] rows (den, num_hi, num_lo) per (pair, t);
    col-group packed 4 pairs per result bank via tile_position=(0, 32j).
  - VectorE flushes banks to SBUF; a DRAM scratch bounce re-lays 32 pairs
    into a dense [32, 1536] tile; VectorE combines partials,
    reciprocal_approx_fast + multiply; one contiguous DMA per block to the
    n-major output.
"""

from contextlib import ExitStack

import numpy as np
import ml_dtypes

import concourse.bacc as bacc
import concourse.bass as bass
import concourse.tile as tile
from concourse import mybir
from concourse.bass_utils import run_bass_kernel_spmd

FP = mybir.dt.float32
BF16 = mybir.dt.bfloat16
AF = mybir.ActivationFunctionType
ALU = mybir.AluOpType

L = 32768          # sequence length
N = 32             # batch
BS = 256           # block size
NB = L // BS       # 128 blocks
NCORES = 8
BPC = NB // NCORES  # 16 blocks per core
LS = BPC * BS       # 4096 rows per core shard

GROUP = 3           # pairs per exp staging group (3 PSUM banks)
PAIRS = BPC * N     # 512 pairs per core
NGRP = (PAIRS + GROUP - 1) // GROUP  # 171 act groups (last has 2 pairs)
SPG = 4             # groups (= slots) per q/k stage DMA
PW = 512            # bf16 cols per pair in the qk stage (256 rhs + 2*128 lhsT)

BF_NP = ml_dtypes.bfloat16


def build_kernel_module(reps: int = 1) -> bass.Bass:
    """reps > 1 wraps the body in a device-side For_i loop (benchmarking)."""
    nc = bacc.Bacc("TRN2", target_bir_lowering=False, debug=False, num_devices=NCORES)
    # Score matmuls are packed 3-at-a-time into PE row groups 0/32/64
    # (tile_position=(32s, 0)); pair 3G+s uses row group s, i.e. SBUF
    # partitions 32s..32s+2.  qkst row r = 3s + c maps to partition 32s + c;
    # col slot = act-group index G.  Per pair slot (cols G*PW ..):
    #   cols 0:256   rows (qhi, qhi, qlo)    -> rhs [3, 256]
    #   cols 256:384 rows (khi0, klo0, khi0) -> lhsT t=0 [3, 128]
    #   cols 384:512 rows (khi1, klo1, khi1) -> lhsT t=1 [3, 128]
    qkst = nc.declare_dram_parameter("qkst", [9, NGRP * PW], BF16, isOutput=False)
    # vtab[p, b*192 + t*96 + n*3 + c]: c = (1.0, vhi, vlo) of v_eff[b*BS+t*128+p, n]
    vtab = nc.declare_dram_parameter("vtab", [128, BPC * 2 * N * 3], BF16, isOutput=False)
    out_t = nc.declare_dram_parameter("out_t", [N, LS], FP, isOutput=True)

    with tile.TileContext(nc) as tc:
        with ExitStack() as ctx:
            if reps == 1:
                _emit(ctx, tc, qkst, vtab, out_t)
            else:
                with tc.For_i(0, reps, 1):
                    _emit(ctx, tc, qkst, vtab, out_t)
    nc.compile()
    return nc


def _emit(ctx, tc, qkst, vtab, out_t):
    nc = tc.nc

    stage = ctx.enter_context(tc.tile_pool(name="stage", bufs=3))
    vpool = ctx.enter_context(tc.tile_pool(name="vpool", bufs=2))
    epool = ctx.enter_context(tc.tile_pool(name="epool", bufs=3))
    dpool = ctx.enter_context(tc.tile_pool(name="dpool", bufs=2))
    rspool = ctx.enter_context(tc.tile_pool(name="rspool", bufs=3))
    ps_stage = ctx.enter_context(tc.tile_pool(name="ps_stage", bufs=2, space="PSUM"))
    ps_res = ctx.enter_context(tc.tile_pool(name="ps_res", bufs=2, space="PSUM"))
    drs = ctx.enter_context(tc.tile_pool(name="drs", bufs=2, space="DRAM"))

    warm = dpool.tile([1, 8], FP, name="warm", tag="warm")
    nc.vector.memset(warm[:], 0.0)
    nc.scalar.activation(warm[:], warm[:], AF.Exp)

    def load_stage(w):
        # SPG slots (= act groups); 9 dram rows -> partitions {32s + c}
        qs = stage.tile([128, SPG * PW], BF16, name="qs", tag="qs")
        w0 = w * SPG * PW
        width = min(SPG * PW, NGRP * PW - w0)
        # NB: one dma per row group — a single strided-partition view write
        # is not reliably ordered against the sliced matmul reads.
        for s in range(3):
            nc.sync.dma_start(
                out=qs[32 * s:32 * s + 3, 0:width],
                in_=qkst[3 * s:3 * s + 3, w0:w0 + width],
            )
        return qs

    def load_vtile(b):
        vt = vpool.tile([128, 2, N, 3], BF16, name="vt", tag="vt")
        nc.sync.dma_start(out=vt[:], in_=vtab[:, b * (2 * N * 3):(b + 1) * (2 * N * 3)])
        return vt

    # --- main loop --------------------------------------------------------------
    vcur = [None]
    res_state = {"tile": None, "count": 0, "nflush": 0, "rs": None, "first_g": 0}

    def emit_reduces(pend):
        e, members = pend
        for (s, b, n, vc) in members:
            g = b * N + n
            r = res_state["count"]
            if r == 0:
                res_state["tile"] = ps_res.tile([128, 256], FP, name="res", tag="res")
                if res_state["nflush"] == 0:
                    res_state["rs"] = rspool.tile([128, 2048], FP, name="rs", tag="rs")
                    res_state["first_g"] = g
            jj = r
            # the two k-halves accumulate in PSUM: rows = (den, num_hi, num_lo)
            for t in (0, 1):
                nc.tensor.matmul(
                    res_state["tile"][32 * jj:32 * jj + 3, 0:256],
                    lhsT=vc[:][:, t, n, :],
                    rhs=e[:][:, s * 512 + t * 256: s * 512 + (t + 1) * 256],
                    start=(t == 0), stop=(t == 1),
                    tile_position=(0, 32 * jj),
                )
            res_state["count"] += 1
            if res_state["count"] == 4:
                m = res_state["nflush"]
                nc.vector.tensor_copy(
                    res_state["rs"][:, m * 256:(m + 1) * 256], res_state["tile"][:]
                )
                res_state["count"] = 0
                res_state["tile"] = None
                res_state["nflush"] += 1
                if res_state["nflush"] == 8:
                    division_batch()

    def division_batch():
        b0 = res_state["first_g"] // N
        rs = res_state["rs"]
        # rows {32j+r} of rs -> DRAM scratch already in dense layout:
        # scr[4m+j, r*256 + q] ; then scratch -> dn is a contiguous copy.
        scr = drs.tile([N, 768], FP, name="scr", tag="scr")
        rsv = rs[:].rearrange("(j p2) (m q) -> j p2 m q", j=4, m=8)
        sw = scr[:].rearrange("(m j) (r q) -> j m r q", m=8, r=3)
        for r in (0, 1, 2):
            nc.gpsimd.dma_start(out=sw[:, :, r, :], in_=rsv[:, r, :, :])
        # scratch -> dense [32, 768]: partition 4m+j (= local pair n), free (r,q)
        dn = dpool.tile([N, 768], FP, name="dn", tag="dn")
        nc.gpsimd.dma_start(out=dn[:], in_=scr[:])
        dnv = dn[:].rearrange("p (r q) -> p r q", r=3)
        den = dpool.tile([N, BS], FP, name="den", tag="den")
        num = dpool.tile([N, BS], FP, name="num", tag="num")
        nc.vector.tensor_add(num[:], dnv[:, 1, :], dnv[:, 2, :])
        nc.vector.reciprocal_approx_fast(out=den[:], in_=dnv[:, 0, :])
        ov = dpool.tile([N, BS], FP, name="ov", tag="ov")
        nc.vector.tensor_mul(ov[:], num[:], den[:])
        nc.gpsimd.dma_start(out=out_t[:, b0 * BS:(b0 + 1) * BS], in_=ov[:])
        res_state["nflush"] = 0
        res_state["rs"] = None

    pending = None
    cur_qs = None
    for G in range(NGRP):
        gpairs = [g for g in range(GROUP * G, min(GROUP * (G + 1), PAIRS))]
        if G % SPG == 0:
            cur_qs = load_stage(G // SPG)
        wi = G % SPG
        qsv = cur_qs[:]
        cur_stage = ps_stage.tile([128, GROUP * 512], FP, name="st", tag="st")
        members = []
        for s, g in enumerate(gpairs):
            if g % N == 0:
                vcur[0] = load_vtile(g // N)
            members.append((s, g // N, g % N, vcur[0]))
        # waves of 3 co-streaming row-group-tiled score matmuls (t-outer:
        # a pair's two halves share a row group and serialize; distinct
        # pairs use distinct row groups AND distinct PSUM banks)
        for t in (0, 1):
            for s, g in enumerate(gpairs):
                nc.tensor.matmul(
                    cur_stage[:, s * 512 + t * 256: s * 512 + (t + 1) * 256],
                    lhsT=qsv[32 * s:32 * s + 3,
                             wi * PW + 256 + t * 128: wi * PW + 256 + (t + 1) * 128],
                    rhs=qsv[32 * s:32 * s + 3, wi * PW: wi * PW + 256],
                    start=True, stop=True,
                    tile_position=(32 * s, 0),
                )
        e = epool.tile([128, GROUP * 512], BF16, name="e", tag="e")
        width = len(members) * 512
        nc.scalar.activation(e[:][:, 0:width], cur_stage[:][:, 0:width], AF.Exp)
        if pending is not None:
            emit_reduces(pending)
        pending = (e, members)
    emit_reduces(pending)
    assert res_state["count"] == 0 and res_state["nflush"] == 0, (
        "pair count must be a multiple of 32 (one block per division batch)"
    )


_CACHE: dict = {}


def _get_nc(reps: int = 1) -> bass.Bass:
    if reps not in _CACHE:
        _CACHE[reps] = build_kernel_module(reps)
    return _CACHE[reps]


def _split_bf16(x):
    hi = x.astype(BF_NP)
    lo = (x - hi.astype(np.float32)).astype(BF_NP)
    return hi, lo


def make_in_maps(query, key, value, in_proj_w, in_proj_b, out_proj_w, out_proj_b):
    q = np.asarray(query, dtype=np.float32).reshape(L, N)
    k = np.asarray(key, dtype=np.float32).reshape(L, N)
    vv = np.asarray(value, dtype=np.float32).reshape(L, N)
    wq, wk, wv = [float(x) for x in np.asarray(in_proj_w, dtype=np.float32).reshape(3)]
    bq, bk, bv = [float(x) for x in np.asarray(in_proj_b, dtype=np.float32).reshape(3)]
    wo = float(np.asarray(out_proj_w, dtype=np.float32).reshape(1)[0])
    bo = float(np.asarray(out_proj_b, dtype=np.float32).reshape(1)[0])

    qp = q * np.float32(wq) + np.float32(bq)
    kp = k * np.float32(wk) + np.float32(bk)
    # softmax weights sum to 1 -> the whole v/out affine folds into v:
    veff = (vv * np.float32(wv) + np.float32(bv)) * np.float32(wo) + np.float32(bo)

    qhi, qlo = _split_bf16(qp)
    khi, klo = _split_bf16(kp)
    vhi, vlo = _split_bf16(veff)

    in_maps = []
    for c in range(NCORES):
        sl = slice(c * LS, (c + 1) * LS)
        # [LS, N] core shards -> per-pair vectors; pair g = b*N + n
        def pairs_of(x):
            # -> [PAIRS, BS] (pair-major), x is [LS, N]
            return np.ascontiguousarray(
                x[sl].reshape(BPC, BS, N).transpose(0, 2, 1).reshape(PAIRS, BS)
            )

        qh, ql = pairs_of(qhi), pairs_of(qlo)
        kh, kl = pairs_of(khi), pairs_of(klo)
        # row 3s+c <-> SBUF partition 32s+c; col slot = act group G; pair 3G+s
        qkst = np.zeros((3, 3, NGRP, PW), dtype=BF_NP)
        for s in range(3):
            sel = slice(s, PAIRS, 3)          # pairs 3G+s
            cnt = len(range(PAIRS)[sel])
            qkst[s, 0, :cnt, 0:256] = qh[sel]
            qkst[s, 1, :cnt, 0:256] = qh[sel]
            qkst[s, 2, :cnt, 0:256] = ql[sel]
            qkst[s, 0, :cnt, 256:512] = kh[sel]
            qkst[s, 1, :cnt, 256:512] = kl[sel]
            qkst[s, 2, :cnt, 256:512] = kh[sel]
        qkst = np.ascontiguousarray(qkst.reshape(9, NGRP * PW))

        # vtab[p, (b, t, n, c)] with c = (1, vhi, vlo)
        vt = np.empty((128, BPC, 2, N, 3), dtype=BF_NP)
        vt[:, :, :, :, 0] = np.float32(1.0)
        # vhi[sl] is [LS, N] = [(b t p), n]
        vt[:, :, :, :, 1] = vhi[sl].reshape(BPC, 2, 128, N).transpose(2, 0, 1, 3)
        vt[:, :, :, :, 2] = vlo[sl].reshape(BPC, 2, 128, N).transpose(2, 0, 1, 3)
        vt = np.ascontiguousarray(vt.reshape(128, BPC * 2 * N * 3))

        in_maps.append({"qkst": qkst, "vtab": vt})
    return in_maps, None


def run(in_maps, sc=None, **kwargs):
    return run_bass_kernel_spmd(_get_nc(), in_maps, list(range(NCORES)), **kwargs)


def assemble(results) -> np.ndarray:
    outs = [np.asarray(results[c]["out_t"], dtype=np.float32).T for c in range(NCORES)]
    return np.ascontiguousarray(np.concatenate(outs, axis=0)).reshape(L, N, 1)


def kernel(query, key, value, in_proj_w, in_proj_b, out_proj_w, out_proj_b):
    in_maps, sc = make_in_maps(
        query, key, value, in_proj_w, in_proj_b, out_proj_w, out_proj_b
    )
    res = run(in_maps, sc)
    return assemble(res.results)
